# revision 1
# baseline (speedup 1.0000x reference)
"""Trainium2 kernel for nn_EulerBias: exact Riemann-solver bias field.

Structure:
  * Host (numpy, float32): the K-interface Newton solve (tiny: B x 63) ->
    wave speeds, then per-batch coefficient matrices for the device stage.
  * Device (8 NeuronCores, batch-parallel, 2 batches/core): for every query
    point q the bias over the 64 segment columns is

        out[q,k] = min(T1[q,k],0) + min(T2[q,k],0)

    where T1/T2 are affine in (u,it,1) = (x*it, 1/(t+eps), 1) with per-k
    coefficients -> one small-contraction (Kc=24) bf16 matmul on TensorE
    produces T1||T2 for 512 queries per instruction.  Operands use a
    double-bf16 split (u_hi+u_lo, it_hi+it_lo; bf16 products are exact in
    the f32 psum accumulate -> ~f32 matmul precision) - bf16 streams at PE
    compute rate where f32r is xbus-bound at ~427ns/512 rows (head-to-head:
    39.2us vs 58.0us same-phase).  ScalarE computes relu(-T2); VectorE fuses
    min(T1,0) - relu(-T2) in one op, writing bf16; DMA stores 512KB
    contiguous bf16 blocks (host upcasts to f32 - bias absmax ~2e5, bf16
    output keeps rel err ~2e-3, gate is 2e-2).

    NOTE: 4-way tile_position=(32g,0) row-group matmul concurrency (best
    measured 25.7us) intermittently hard-hangs the exec unit
    (NRT_EXEC_UNIT_UNRECOVERABLE, ~50% of runs, f32r and bf16, Kc=24 and 32)
    and must not be used; this kernel keeps serial matmuls at base
    partition 0 - the vanilla path, stable across all runs.

Masked columns (pieces_mask == 0) are encoded in the coefficients
(T1 = -1e9, T2 = +1e30) so no separate mask pass is needed. Assumes
pieces_mask >= 0 (it is a 0/1 mask; the harness fills ones).
"""

import numpy as np

GAMMA = np.float32(1.4)
EPS = np.float32(1e-6)
N_NEWTON = 20
B, K, NT, NX = 16, 64, 128, 256
NQ = NT * NX            # 32768 queries per batch
N_CORES = 8
B_PER_CORE = B // N_CORES
# device tiling: query q = sp*4096 + m*32 + h*16 + g*4 + j
#   m: psum partition (128), h: psum-tile half (2), g: matmul-in-half (4),
#   j: contraction group (4).  One sp-iteration = 4096 queries.
CHUNK = 128
GROUPS = 4
SP_Q = 4096                      # queries per sp iteration
SP_PER_BATCH = NQ // SP_Q        # 8
BIG = np.float32(1e30)
NEGBIG = np.float32(-1e9)

_COMPILED = {}


def _f32(x):
    return np.asarray(x, dtype=np.float32)


def _host_wave_speeds(xs, ks, ks_v, ks_p):
    """Mirror of reference.py's f32 Newton solve, in numpy float32."""
    gm1 = np.float32(GAMMA - 1.0)
    gp1 = np.float32(GAMMA + 1.0)
    exp_rare = np.float32(gm1 / (2.0 * GAMMA))

    def clip_lo(v, lo=EPS):
        return np.maximum(v, lo)

    rho_L, rho_R = ks[:, :-1], ks[:, 1:]
    u_L, u_R = ks_v[:, :-1], ks_v[:, 1:]
    p_L, p_R = ks_p[:, :-1], ks_p[:, 1:]

    def sound(rho, p):
        return np.sqrt(clip_lo(GAMMA * p / clip_lo(rho)))

    c_L, c_R = sound(rho_L, p_L), sound(rho_R, p_R)
    A_L = np.float32(2.0) / (gp1 * clip_lo(rho_L))
    A_R = np.float32(2.0) / (gp1 * clip_lo(rho_R))
    B_L = gm1 / gp1 * p_L
    B_R = gm1 / gp1 * p_R

    def wave_f_df(p, p_K, A_K, B_K, c_K):
        denom = clip_lo(p + B_K)
        sqrt_AoD = np.sqrt(clip_lo(A_K / denom))
        f_shock = (p - p_K) * sqrt_AoD
        df_shock = sqrt_AoD * (np.float32(1.0) - (p - p_K) / (np.float32(2.0) * denom))
        p_ratio = clip_lo(p / clip_lo(p_K))
        f_rare = np.float32(2.0) * c_K / gm1 * (p_ratio ** exp_rare - np.float32(1.0))
        df_rare = c_K / (GAMMA * clip_lo(p_K)) * p_ratio ** np.float32(-gp1 / (2.0 * GAMMA))
        is_shock = p > p_K
        return np.where(is_shock, f_shock, f_rare), np.where(is_shock, df_shock, df_rare)

    p0 = clip_lo(((c_L + c_R - gm1 / np.float32(2.0) * (u_R - u_L))
                  / (c_L / clip_lo(p_L) ** exp_rare + c_R / clip_lo(p_R) ** exp_rare))
                 ** np.float32(1.0 / exp_rare))
    p_star = p0
    for _ in range(N_NEWTON):
        f_L, df_L = wave_f_df(p_star, p_L, A_L, B_L, c_L)
        f_R, df_R = wave_f_df(p_star, p_R, A_R, B_R, c_R)
        residual = f_L + f_R + (u_R - u_L)
        jacobian = clip_lo(df_L + df_R)
        p_star = clip_lo(p_star - residual / jacobian)

    gp1_o_2g = np.float32(gp1 / (2.0 * GAMMA))
    sigma_1 = u_L - c_L * np.sqrt(clip_lo(np.float32(1.0) + gp1_o_2g * (p_star / clip_lo(p_L) - np.float32(1.0))))
    speed_left = np.where(p_star > p_L, sigma_1, u_L - c_L)
    sigma_3 = u_R + c_R * np.sqrt(clip_lo(np.float32(1.0) + gp1_o_2g * (p_star / clip_lo(p_R) - np.float32(1.0))))
    speed_right = np.where(p_star > p_R, sigma_3, u_R + c_R)
    return speed_left.astype(np.float32), speed_right.astype(np.float32)


def _host_coef(xs, mask, sl, sr):
    """Per-batch [12, 512] coefficient matrices (the matmul's moving operand).

    psum col n = 64*j + k       (j = contraction group) -> T1
    psum col n = 256 + 64*j + k                         -> T2
    contraction rows 3j+(0,1,2) multiply (u, it, 1) of group j.
    """
    xd = xs[:, 1:K]                      # (B, 63)
    m = mask.astype(np.float32)          # (B, 64)
    act = m != 0

    # T1 = -m*u + m*xd*it + m*sr   (k < 63);  col 63 -> +BIG;  masked -> -1e9
    Wu1 = np.zeros((B, K), np.float32)
    Wi1 = np.zeros((B, K), np.float32)
    Wc1 = np.zeros((B, K), np.float32)
    Wu1[:, :63] = -m[:, :63]
    Wi1[:, :63] = m[:, :63] * xd
    Wc1[:, :63] = m[:, :63] * sr
    Wc1[:, 63] = BIG
    Wu1[~act] = 0.0
    Wi1[~act] = 0.0
    Wc1[~act] = NEGBIG

    # T2 = m*u - m*xd[k-1]*it - m*sl[k-1] (k >= 1); col 0 or masked -> +BIG
    # (so min(T2,0) = -m*relu(sl[k-1] - xi[k-1]))
    Wu2 = np.zeros((B, K), np.float32)
    Wi2 = np.zeros((B, K), np.float32)
    Wc2 = np.zeros((B, K), np.float32)
    Wu2[:, 1:] = m[:, 1:]
    Wi2[:, 1:] = -m[:, 1:] * xd
    Wc2[:, 1:] = -m[:, 1:] * sl
    Wc2[:, 0] = BIG
    Wu2[~act] = 0.0
    Wi2[~act] = 0.0
    Wc2[~act] = BIG

    # double-bf16 contraction: xd*it = xd_hi*it_hi + xd_lo*it_hi + xd_hi*it_lo
    # (+O(2^-18) dropped), u = u_hi + u_lo.  bf16 x bf16 products are exact in
    # the f32 psum accumulate -> ~f32 precision with 2-byte operand streaming.
    # 6 contraction rows per group j: [u_hi, u_lo, it_hi, it_hi, it_lo, 1]
    import ml_dtypes
    bf = ml_dtypes.bfloat16
    xd_hi = xd.astype(bf).astype(np.float32)
    xd_lo = (xd - xd_hi).astype(bf).astype(np.float32)
    Wi1_hi = np.zeros((B, K), np.float32); Wi1_hi[:, :63] = m[:, :63] * xd_hi
    Wi1_lo = np.zeros((B, K), np.float32); Wi1_lo[:, :63] = m[:, :63] * xd_lo
    Wi2_hi = np.zeros((B, K), np.float32); Wi2_hi[:, 1:] = -m[:, 1:] * xd_hi
    Wi2_lo = np.zeros((B, K), np.float32); Wi2_lo[:, 1:] = -m[:, 1:] * xd_lo
    for W in (Wi1_hi, Wi1_lo, Wi2_hi, Wi2_lo):
        W[~act] = 0.0
    rows1 = [Wu1, Wu1, Wi1_hi, Wi1_lo, Wi1_hi, Wc1]
    rows2 = [Wu2, Wu2, Wi2_hi, Wi2_lo, Wi2_hi, Wc2]
    NR = 6
    coef = np.zeros((B, NR * GROUPS, 512), np.float32)
    for j in range(GROUPS):
        c1 = slice(64 * j, 64 * j + 64)
        c2 = slice(256 + 64 * j, 256 + 64 * j + 64)
        for r in range(NR):
            coef[:, NR * j + r, c1] = rows1[r]
            coef[:, NR * j + r, c2] = rows2[r]
    return coef.astype(bf)


def _host_qdata(t_coords, x_coords):
    """(B, SP, 4, 12, 256) stationary operands for query
    q = sp*4096 + m*32 + h*16 + g*4 + j: rows 6j+(0..5) =
    (u_hi, u_lo, it_hi, it_hi, it_lo, 1), column 512*h + 128*g + m.

    The m-major query assignment makes each sp-iteration's store one
    contiguous 512KB bf16 HBM range (4KB per partition row)."""
    it = np.float32(1.0) / (t_coords.reshape(B, NQ) + EPS)
    u = x_coords.reshape(B, NQ) * it

    def lay(v):
        # (b, sp, m, h, g, j) -> [b, sp, j, (h, g, m)]
        v = v.reshape(B, SP_PER_BATCH, CHUNK, 2, GROUPS, GROUPS)
        return np.transpose(v, (0, 1, 5, 3, 4, 2)).reshape(
            B, SP_PER_BATCH, GROUPS, 2 * GROUPS * CHUNK)

    import ml_dtypes
    bf = ml_dtypes.bfloat16
    u_hi = u.astype(bf).astype(np.float32)
    u_lo = (u - u_hi).astype(np.float32)
    it_hi = it.astype(bf).astype(np.float32)
    it_lo = (it - it_hi).astype(np.float32)
    NR = 6
    qd = np.empty((B, SP_PER_BATCH, NR * GROUPS, 2 * GROUPS * CHUNK), np.float32)
    for r, v in ((0, u_hi), (1, u_lo), (2, it_hi), (3, it_hi), (4, it_lo)):
        qd[:, :, r::NR, :] = lay(v)
    qd[:, :, 5::NR, :] = 1.0
    return qd.astype(bf)


def _build_nc(repeat=1):
    import concourse.bacc as bacc
    import concourse.mybir as mybir
    import concourse.tile as tile

    nc = bacc.Bacc(None, target_bir_lowering=False, debug=False)
    f32r = mybir.dt.float32r
    f32 = mybir.dt.float32
    bf16 = mybir.dt.bfloat16

    qd_d = nc.declare_dram_parameter(
        "qd", [B_PER_CORE, SP_PER_BATCH, 6 * GROUPS, 2 * GROUPS * CHUNK],
        bf16, isOutput=False)
    cf_d = nc.declare_dram_parameter(
        "cf", [B_PER_CORE, 6 * GROUPS, 512], bf16, isOutput=False)
    out_d = nc.declare_dram_parameter(
        "out", [B_PER_CORE, NQ, K], bf16, isOutput=True)

    with tile.TileContext(nc) as tc:
        with (
            tc.tile_pool(name="cf", bufs=1) as cfp,
            tc.tile_pool(name="qd", bufs=6) as qdp,
            tc.tile_pool(name="ps", bufs=4, space="PSUM") as psp,
            tc.tile_pool(name="p2", bufs=8) as p2p,
            tc.tile_pool(name="ot", bufs=6) as otp,
        ):
            cft = []
            for b in range(B_PER_CORE):
                c = cfp.tile([6 * GROUPS, 512], bf16, tag=f"cf{b}")
                nc.sync.dma_start(c[:], cf_d[b])
                cft.append(c)
            n_iter = 0
            for _ in range(repeat):
                for b in range(B_PER_CORE):
                    for sp in range(SP_PER_BATCH):
                        qdt = qdp.tile([6 * GROUPS, 2 * GROUPS * CHUNK], bf16)
                        # first load on the (empty) ACT HWDGE ring, parallel
                        # with cf on the SP ring: first matmul ~2us earlier
                        eng = nc.scalar if n_iter == 0 else nc.gpsimd
                        eng.dma_start(qdt[:], qd_d[b, sp])
                        n_iter += 1
                        ot = otp.tile([128, 2, GROUPS, 256], bf16)
                        # 2-bank psum tiles, 4 in rotation: the psum-reuse
                        # dependency cycle DVE(t) -> MM(t+4) -> ACT -> DVE
                        # amortizes over 4 tile-slots instead of 2
                        for h in range(2):
                            for gp in range(2):
                                ps = psp.tile([128, 2, 512], f32, name="ps")
                                if n_iter == 1 and h == 0 and gp == 0:
                                    # PE clock-gate warmup: garbage matmul off
                                    # the cf tile as soon as it lands, into a
                                    # bank the first real matmul overwrites -
                                    # starts the HAM ramp during the qd load
                                    nc.tensor.matmul(
                                        ps[:, 0, :], cft[b][:, 0:128],
                                        cft[b][:],
                                        start=True, stop=True)
                                for g2 in range(2):
                                    g = 2 * gp + g2
                                    nc.tensor.matmul(
                                        ps[:, g2, :],
                                        qdt[:, 512 * h + 128 * g:512 * h + 128 * (g + 1)],
                                        cft[b][:],
                                        start=True, stop=True,
                                    )
                                p2 = p2p.tile([128, 2, 256], bf16)
                                nc.scalar.activation(
                                    p2[:], ps[:, :, 256:512],
                                    mybir.ActivationFunctionType.Relu, scale=-1.0)
                                nc.vector.scalar_tensor_tensor(
                                    out=ot[:, h, 2 * gp:2 * gp + 2],
                                    in0=ps[:, :, 0:256], scalar=0.0, in1=p2[:],
                                    op0=mybir.AluOpType.min,
                                    op1=mybir.AluOpType.subtract)
                        q0 = sp * SP_Q
                        dst = out_d[b, q0:q0 + SP_Q, :].rearrange(
                            "(m c) k -> m (c k)", c=32)
                        src = ot[:].rearrange("m h g x -> m (h g x)")
                        n_total = repeat * B_PER_CORE * SP_PER_BATCH
                        if n_iter == n_total:
                            # final store split across both HWDGE rings (all
                            # activations are done by then): halves the
                            # end-of-kernel drain
                            nc.scalar.dma_start(dst[:, 0:1024], src[:, 0:1024])
                            nc.sync.dma_start(dst[:, 1024:2048], src[:, 1024:2048])
                        elif n_iter % 3 == 0:
                            # every 3rd store via SWDGE: keeps the SP ring
                            # (16 x 2.6us would be exactly DVE-co-critical)
                            # comfortably under the DVE period
                            nc.gpsimd.dma_start(dst, src)
                        else:
                            nc.sync.dma_start(dst, src)
    nc.compile()
    return nc


def _get_compiled(repeat=1):
    if repeat not in _COMPILED:
        _COMPILED[repeat] = _build_nc(repeat)
    return _COMPILED[repeat]


def _prep_inputs(inputs):
    xs = _f32(inputs["xs"])
    ks = _f32(inputs["ks"])
    ks_v = _f32(inputs["ks_v"])
    ks_p = _f32(inputs["ks_p"])
    mask = _f32(inputs["pieces_mask"])
    t_coords = _f32(inputs["t_coords"])
    x_coords = _f32(inputs["x_coords"])

    sl, sr = _host_wave_speeds(xs, ks, ks_v, ks_p)
    coef = _host_coef(xs, mask, sl, sr)
    qd = _host_qdata(t_coords, x_coords)
    return [
        {
            "qd": np.ascontiguousarray(qd[c * B_PER_CORE:(c + 1) * B_PER_CORE]),
            "cf": np.ascontiguousarray(coef[c * B_PER_CORE:(c + 1) * B_PER_CORE]),
        }
        for c in range(N_CORES)
    ]


def run(inputs, trace=False):
    from concourse.bass_utils import run_bass_kernel_spmd

    in_maps = _prep_inputs(inputs)
    nc = _get_compiled()
    res = None
    for attempt in range(3):
        try:
            res = run_bass_kernel_spmd(
                nc, in_maps, core_ids=list(range(N_CORES)), trace=trace)
            break
        except Exception:
            if attempt == 2:
                raise
            import time as _time
            _time.sleep(2.0)
    out = np.empty((B, NT, NX, K), np.float32)
    for c in range(N_CORES):
        out[c * B_PER_CORE:(c + 1) * B_PER_CORE] = (
            res.results[c]["out"].astype(np.float32).reshape(B_PER_CORE, NT, NX, K))
    return out, res


def kernel(**inputs):
    out, _ = run(inputs, trace=False)
    return out



# revision 2
# speedup vs baseline: 1.6036x; 1.6036x over previous
"""Trainium2 kernel for nn_EulerBias: exact Riemann-solver bias field.

Host (numpy f32): Newton solve for wave speeds -> per-batch coefficient
matrices.  Device (8 cores, 2 batches each): for query q and column k

    -bias[q,k] = relu(S1_k(q)) + relu(S2_{k-1}(q))
    S1_k = xi_k - sr_k   (k<63; col 63 -> -BIG)
    S2_i = sl_i - xi_i   (shifted into col i+1; col 0 -> -BIG)
    xi_i = x*it - xd_i*it,  it = 1/(t+eps)

Both S1 and S2 are affine in the 6 query features
(u_hi, u_lo, it_hi, it_hi, it_lo, 1) (double-bf16 split for ~f32 precision),
so one bf16 matmul per 1024-query block produces each 512-col psum bank
(8 column-groups of 64 k per bank; 2 banks per psum tile -> FD=1024
elementwise ops, which amortizes the per-instruction init overhead that
made the FD=512 version DVE-bound).

Engine split (cost-model balanced):
  ACT: p2 = relu(S2) psum->bf16            (~1.04 ns/col)
  DVE: out = max(S1,0) + p2  (stt, fused)  (~1.16 ns/col)
  every ~14th pair-iter instead: ACT does both relus and GpSimd adds them
  (bf16 TT), shifting work off the critical DVE stream.
Stores (bf16, -bias; host negates during the f32 upcast): SP HWDGE ring
for most, SWDGE (gpsimd) for the rest -- never the scalar ring mid-kernel
(ACT's exec queue depth is 0; DMA issue there stalls activations).
"""

import contextlib

import numpy as np


@contextlib.contextmanager
def _no_auto_ldw():
    import concourse.mybir as mybir
    orig = mybir.InstMatmult

    def patched(**kw):
        kw['ldweights'] = False
        return orig(**kw)

    mybir.InstMatmult = patched
    try:
        yield
    finally:
        mybir.InstMatmult = orig


GAMMA = np.float32(1.4)
EPS = np.float32(1e-6)
N_NEWTON = 20
B, K, NT, NX = 16, 64, 128, 256
NQ = NT * NX            # 32768 queries per batch
N_CORES = 8
B_PER_CORE = B // N_CORES
# query q = sp*4096 + m*32 + u*8 + j
#   m: psum partition (128), u: block-in-sp (4), j: column group (8)
GROUPS = 8
NBLK = 4                 # blocks per sp-iter
SP_Q = 4096
SP_PER_BATCH = NQ // SP_Q        # 8
NR = 6                   # contraction rows per group
KC = NR * GROUPS         # 48 contraction rows
BIG = np.float32(1e30)

_COMPILED = {}


def _f32(x):
    return np.asarray(x, dtype=np.float32)


def _host_wave_speeds(xs, ks, ks_v, ks_p):
    """Mirror of reference.py's f32 Newton solve, in numpy float32."""
    gm1 = np.float32(GAMMA - 1.0)
    gp1 = np.float32(GAMMA + 1.0)
    exp_rare = np.float32(gm1 / (2.0 * GAMMA))

    def clip_lo(v, lo=EPS):
        return np.maximum(v, lo)

    rho_L, rho_R = ks[:, :-1], ks[:, 1:]
    u_L, u_R = ks_v[:, :-1], ks_v[:, 1:]
    p_L, p_R = ks_p[:, :-1], ks_p[:, 1:]

    def sound(rho, p):
        return np.sqrt(clip_lo(GAMMA * p / clip_lo(rho)))

    c_L, c_R = sound(rho_L, p_L), sound(rho_R, p_R)
    A_L = np.float32(2.0) / (gp1 * clip_lo(rho_L))
    A_R = np.float32(2.0) / (gp1 * clip_lo(rho_R))
    B_L = gm1 / gp1 * p_L
    B_R = gm1 / gp1 * p_R

    def wave_f_df(p, p_K, A_K, B_K, c_K):
        denom = clip_lo(p + B_K)
        sqrt_AoD = np.sqrt(clip_lo(A_K / denom))
        f_shock = (p - p_K) * sqrt_AoD
        df_shock = sqrt_AoD * (np.float32(1.0) - (p - p_K) / (np.float32(2.0) * denom))
        p_ratio = clip_lo(p / clip_lo(p_K))
        f_rare = np.float32(2.0) * c_K / gm1 * (p_ratio ** exp_rare - np.float32(1.0))
        df_rare = c_K / (GAMMA * clip_lo(p_K)) * p_ratio ** np.float32(-gp1 / (2.0 * GAMMA))
        is_shock = p > p_K
        return np.where(is_shock, f_shock, f_rare), np.where(is_shock, df_shock, df_rare)

    p0 = clip_lo(((c_L + c_R - gm1 / np.float32(2.0) * (u_R - u_L))
                  / (c_L / clip_lo(p_L) ** exp_rare + c_R / clip_lo(p_R) ** exp_rare))
                 ** np.float32(1.0 / exp_rare))
    p_star = p0
    for _ in range(N_NEWTON):
        f_L, df_L = wave_f_df(p_star, p_L, A_L, B_L, c_L)
        f_R, df_R = wave_f_df(p_star, p_R, A_R, B_R, c_R)
        residual = f_L + f_R + (u_R - u_L)
        jacobian = clip_lo(df_L + df_R)
        p_star = clip_lo(p_star - residual / jacobian)

    gp1_o_2g = np.float32(gp1 / (2.0 * GAMMA))
    sigma_1 = u_L - c_L * np.sqrt(clip_lo(np.float32(1.0) + gp1_o_2g * (p_star / clip_lo(p_L) - np.float32(1.0))))
    speed_left = np.where(p_star > p_L, sigma_1, u_L - c_L)
    sigma_3 = u_R + c_R * np.sqrt(clip_lo(np.float32(1.0) + gp1_o_2g * (p_star / clip_lo(p_R) - np.float32(1.0))))
    speed_right = np.where(p_star > p_R, sigma_3, u_R + c_R)
    return speed_left.astype(np.float32), speed_right.astype(np.float32)


def _host_coef(xs, mask, sl, sr):
    """Per-batch [2, KC, 512] coefficient matrices (S1-matrix, S2-matrix).

    psum col n = 64*j + k (j = column group).  Contraction rows 6j+(0..5)
    multiply features (u_hi, u_lo, it_hi, it_hi, it_lo, 1) of group j's
    query.  S1 col k = xi_k - sr_k (k<63; else -BIG); S2 col k holds the
    interface k-1 term sl_{k-1} - xi_{k-1} (k>=1; col 0 -> -BIG).
    Masked col k: S1 -> +1e9 const, S2 -> -BIG (out=+1e9; host flips sign).
    """
    import ml_dtypes
    bf = ml_dtypes.bfloat16

    xd = xs[:, 1:K]                      # (B, 63) interface positions
    m = mask.astype(np.float32)          # (B, 64)
    act = m != 0
    xd_hi = xd.astype(bf).astype(np.float32)
    xd_lo = (xd - xd_hi).astype(bf).astype(np.float32)

    # S1 = u - xd_k*it - sr_k  (k < 63)
    W1u = np.zeros((B, K), np.float32)
    W1h = np.zeros((B, K), np.float32)   # coef on it_hi (hi part)
    W1l = np.zeros((B, K), np.float32)   # coef on it_hi (lo part)
    W1c = np.zeros((B, K), np.float32)
    W1u[:, :63] = m[:, :63]
    W1h[:, :63] = -m[:, :63] * xd_hi
    W1l[:, :63] = -m[:, :63] * xd_lo
    W1c[:, :63] = -m[:, :63] * sr
    W1c[:, 63] = -BIG
    W1u[~act] = 0.0
    W1h[~act] = 0.0
    W1l[~act] = 0.0
    W1c[~act] = np.float32(1e9)

    # S2 col k = -u + xd_{k-1}*it + sl_{k-1}  (k >= 1)
    W2u = np.zeros((B, K), np.float32)
    W2h = np.zeros((B, K), np.float32)
    W2l = np.zeros((B, K), np.float32)
    W2c = np.zeros((B, K), np.float32)
    W2u[:, 1:] = -m[:, 1:]
    W2h[:, 1:] = m[:, 1:] * xd_hi
    W2l[:, 1:] = m[:, 1:] * xd_lo
    W2c[:, 1:] = m[:, 1:] * sl
    W2c[:, 0] = -BIG
    W2u[~act] = 0.0
    W2h[~act] = 0.0
    W2l[~act] = 0.0
    W2c[~act] = -BIG

    # rows per group: coefs on (u_hi, u_lo, it_hi, it_hi, it_lo, 1)
    rows1 = [W1u, W1u, W1h, W1l, W1h, W1c]
    rows2 = [W2u, W2u, W2h, W2l, W2h, W2c]
    coef = np.zeros((B, 2, KC, 512), np.float32)
    for j in range(GROUPS):
        c = slice(64 * j, 64 * j + 64)
        for r in range(NR):
            coef[:, 0, NR * j + r, c] = rows1[r]
            coef[:, 1, NR * j + r, c] = rows2[r]
    # (B, KC, 2, 512): DRAM layout must match the [KC, 2, 512] tile order
    return np.ascontiguousarray(np.transpose(coef, (0, 2, 1, 3))).astype(bf)


def _host_qdata(t_coords, x_coords):
    """(B, SP, NBLK, KC, 128) stationary blocks: block u, row 6j+r, col m =
    feature r of query q = sp*4096 + m*32 + u*8 + j."""
    import ml_dtypes
    bf = ml_dtypes.bfloat16

    it = np.float32(1.0) / (t_coords.reshape(B, NQ) + EPS)
    u = x_coords.reshape(B, NQ) * it
    u_hi = u.astype(bf).astype(np.float32)
    u_lo = (u - u_hi).astype(np.float32)
    it_hi = it.astype(bf).astype(np.float32)
    it_lo = (it - it_hi).astype(np.float32)

    def lay(v):
        # (b, sp, m, u, j) -> (b, sp, u, j, m)
        v = v.reshape(B, SP_PER_BATCH, 128, NBLK, GROUPS)
        return np.transpose(v, (0, 1, 3, 4, 2))

    qd = np.empty((B, SP_PER_BATCH, NBLK, KC, 128), np.float32)
    feats = (u_hi, u_lo, it_hi, it_hi, it_lo)
    for r, v in enumerate(feats):
        qd[:, :, :, r::NR, :] = lay(v)
    qd[:, :, :, 5::NR, :] = 1.0
    # (B, SP, KC, NBLK, 128): DRAM layout must match the [KC, NBLK, 128] tile
    return np.ascontiguousarray(np.transpose(qd, (0, 1, 3, 2, 4))).astype(bf)


def _build_nc(repeat=1):
    import os

    import concourse.bacc as bacc
    import concourse.mybir as mybir
    import concourse.tile as tile

    nc = bacc.Bacc(None, target_bir_lowering=False, debug=False)
    f32 = mybir.dt.float32
    bf16 = mybir.dt.bfloat16

    qd_d = nc.declare_dram_parameter(
        "qd", [B_PER_CORE, SP_PER_BATCH, KC, NBLK, 128], bf16, isOutput=False)
    cf_d = nc.declare_dram_parameter(
        "cf", [B_PER_CORE, KC, 2, 512], bf16, isOutput=False)
    out_d = nc.declare_dram_parameter(
        "out", [B_PER_CORE, NQ, K], bf16, isOutput=True)

    n_total = repeat * B_PER_CORE * SP_PER_BATCH * 2   # pair-iters
    NOLDW = 0
    RELIEF_ENG = 'dve'
    # every 5th pair-iter: ACT does both relus, DVE only the bf16 TT add --
    # balances the ACT (~1.0us/iter) and DVE (~1.4us stt) streams
    POOL_EVERY = 5

    with tile.TileContext(nc) as tc:
        with (
            tc.tile_pool(name="cf", bufs=1) as cfp,
            tc.tile_pool(name="qd", bufs=3) as qdp,
            tc.tile_pool(name="ps1", bufs=2, space="PSUM") as ps1p,
            tc.tile_pool(name="ps2", bufs=2, space="PSUM") as ps2p,
            tc.tile_pool(name="p2", bufs=6) as p2p,
            tc.tile_pool(name="ot", bufs=6) as otp,
        ):
            cft = []
            for b in range(B_PER_CORE):
                c = cfp.tile([KC, 2, 512], bf16, tag=f"cf{b}")
                # scalar ring is idle at t=0; steady-state never uses it
                nc.scalar.dma_start(c[:], cf_d[b])
                cft.append(c)
            it_n = 0
            for _ in range(repeat):
                for b in range(B_PER_CORE):
                    for sp in range(SP_PER_BATCH):
                        qdt = qdp.tile([KC, NBLK, 128], bf16)
                        eng = nc.scalar if it_n == 0 else nc.sync
                        eng.dma_start(qdt[:], qd_d[b, sp])
                        for pair in range(2):
                            ps1 = ps1p.tile([128, 2, 512], f32, name="ps1")
                            ps2 = ps2p.tile([128, 2, 512], f32, name="ps2")
                            if it_n == 0 and pair == 0:
                                # PE clock-gate warmup off the cf tile
                                nc.tensor.matmul(
                                    ps1[:, 0, :], cft[b][:, 0, 0:128],
                                    cft[b][:, 0, :], start=True, stop=True)
                            for v in range(2):
                                u = 2 * pair + v
                                if NOLDW:
                                    nc.tensor.ldweights(qdt[:, u, :])
                                    with _no_auto_ldw():
                                        nc.tensor.matmul(
                                            ps2[:, v, :], qdt[:, u, :],
                                            cft[b][:, 1, :],
                                            start=True, stop=True)
                                        nc.tensor.matmul(
                                            ps1[:, v, :], qdt[:, u, :],
                                            cft[b][:, 0, :],
                                            start=True, stop=True)
                                else:
                                    nc.tensor.matmul(
                                        ps2[:, v, :], qdt[:, u, :], cft[b][:, 1, :],
                                        start=True, stop=True)
                                    nc.tensor.matmul(
                                        ps1[:, v, :], qdt[:, u, :], cft[b][:, 0, :],
                                        start=True, stop=True)
                            ot = otp.tile([128, 1024], bf16)
                            p2 = p2p.tile([128, 1024], bf16)
                            if it_n % POOL_EVERY == POOL_EVERY - 1:
                                # DVE-relief form: both relus on ACT, add on
                                # gpsimd (bf16 TT)
                                p1 = p2p.tile([128, 1024], bf16)
                                nc.scalar.activation(
                                    p2[:].rearrange("m (v x) -> m v x", v=2),
                                    ps2[:],
                                    mybir.ActivationFunctionType.Relu)
                                nc.scalar.activation(
                                    p1[:].rearrange("m (v x) -> m v x", v=2),
                                    ps1[:],
                                    mybir.ActivationFunctionType.Relu)
                                eng_tt = RELIEF_ENG
                                if eng_tt == 'mix':
                                    eng_tt = 'pool' if (it_n // RELIEF) % 2 else 'dve'
                                if eng_tt == 'pool':
                                    nc.gpsimd.tensor_tensor(
                                        out=ot[:], in0=p1[:], in1=p2[:],
                                        op=mybir.AluOpType.add)
                                else:
                                    nc.vector.tensor_tensor(
                                        out=ot[:], in0=p1[:], in1=p2[:],
                                        op=mybir.AluOpType.add)
                            else:
                                nc.scalar.activation(
                                    p2[:].rearrange("m (v x) -> m v x", v=2),
                                    ps2[:],
                                    mybir.ActivationFunctionType.Relu)
                                nc.vector.scalar_tensor_tensor(
                                    out=ot[:].rearrange("m (v x) -> m v x", v=2),
                                    in0=ps1[:], scalar=0.0,
                                    in1=p2[:].rearrange("m (v x) -> m v x", v=2),
                                    op0=mybir.AluOpType.max,
                                    op1=mybir.AluOpType.add)
                            q0 = sp * SP_Q
                            dst = out_d[b, q0:q0 + SP_Q, :].rearrange(
                                "(m c) k -> m (c k)", c=32)[
                                :, pair * 1024:(pair + 1) * 1024]
                            it_n += 1
                            if it_n == n_total:
                                # final store split across both HWDGE rings
                                nc.scalar.dma_start(dst[:, 0:512], ot[:, 0:512])
                                nc.sync.dma_start(dst[:, 512:1024], ot[:, 512:1024])
                            elif it_n % 8 in (2, 5, 7):
                                # ~3/8 of stores via SWDGE keeps the SP ring
                                # under the DVE period
                                nc.gpsimd.dma_start(dst, ot[:])
                            else:
                                nc.sync.dma_start(dst, ot[:])
    nc.compile()
    return nc


def _get_compiled(repeat=1):
    if repeat not in _COMPILED:
        _COMPILED[repeat] = _build_nc(repeat)
    return _COMPILED[repeat]


def _prep_inputs(inputs):
    xs = _f32(inputs["xs"])
    ks = _f32(inputs["ks"])
    ks_v = _f32(inputs["ks_v"])
    ks_p = _f32(inputs["ks_p"])
    mask = _f32(inputs["pieces_mask"])
    t_coords = _f32(inputs["t_coords"])
    x_coords = _f32(inputs["x_coords"])

    sl, sr = _host_wave_speeds(xs, ks, ks_v, ks_p)
    coef = _host_coef(xs, mask, sl, sr)
    qd = _host_qdata(t_coords, x_coords)
    return [
        {
            "qd": np.ascontiguousarray(qd[c * B_PER_CORE:(c + 1) * B_PER_CORE]),
            "cf": np.ascontiguousarray(coef[c * B_PER_CORE:(c + 1) * B_PER_CORE]),
        }
        for c in range(N_CORES)
    ]


def run(inputs, trace=False):
    from concourse.bass_utils import run_bass_kernel_spmd

    in_maps = _prep_inputs(inputs)
    nc = _get_compiled()
    res = None
    for attempt in range(3):
        try:
            res = run_bass_kernel_spmd(
                nc, in_maps, core_ids=list(range(N_CORES)), trace=trace)
            break
        except Exception:
            if attempt == 2:
                raise
            import time as _time
            _time.sleep(2.0)
    out = np.empty((B, NT, NX, K), np.float32)
    for c in range(N_CORES):
        # device stores -bias; negate during the f32 upcast
        out[c * B_PER_CORE:(c + 1) * B_PER_CORE] = -(
            res.results[c]["out"].astype(np.float32).reshape(B_PER_CORE, NT, NX, K))
    return out, res


def kernel(**inputs):
    out, _ = run(inputs, trace=False)
    return out
